# revision 19
# baseline (speedup 1.0000x reference)
"""Trainium2 Bass kernel for causal multi-head differential attention.

Reference semantics (per batch b):
  Q = x @ Wq.T -> [L, 2*NH, 32], K likewise, V = x @ Wv.T -> [L, NH, 64]
  scores = Q K^T / sqrt(32), causal-masked, softmax
  lambda_full = exp(lq1.lk1) - exp(lq2.lk2) + 0.2
  A = p_even - lambda_full * p_odd            (per V-head)
  O = rmsnorm(A @ V) * (1 - 0.2);  out = O @ Wo.T
Returns (out [B,L,E], A [B,NH,L,L]).

Sharding: 8 cores = 2 batches x 4 head-groups. Each core owns one batch and
4 V-heads (8 paired Q/K heads), computes its A shard and a partial o_proj
output; the host concatenates A shards and sums the 4 o_proj partials per
batch (tensor-parallel unshard).
"""

import math
import sys

import numpy as np


def _ensure_paths():
    try:
        import concourse.bass  # noqa: F401
        return
    except ImportError:
        pass
    for p in (
        "/root/.axon_site",
        "/root/.axon_site/_ro/trn_rl_repo",
        "/root/.axon_site/_ro/pypackages",
        "/opt/trn_rl_repo",
    ):
        if p not in sys.path:
            sys.path.append(p)


_ensure_paths()

from contextlib import ExitStack

import concourse.bass as bass  # noqa: E402
import concourse.tile as tile  # noqa: E402
from concourse import bacc, mybir  # noqa: E402
from concourse.bass_utils import run_bass_kernel_spmd  # noqa: E402
from concourse.masks import make_identity  # noqa: E402

B, L, E = 2, 2048, 1024
NH = 16
HD = 64
HALF = 32
LAMBDA_INIT = 0.2
SCALE = 1.0 / math.sqrt(HALF)
N_CORES = 8
HPC = 4          # V-heads per core
CPC = HPC * HD   # channels per core (256)

F32 = mybir.dt.float32
BF16 = mybir.dt.bfloat16
AF = mybir.ActivationFunctionType
OP = mybir.AluOpType

TRACE = False
LAST_EXEC_NS = None

_CACHE = {}


def _build():
    if "nc" in _CACHE:
        return _CACHE["nc"]

    nc = bacc.Bacc(None)

    x_d = nc.dram_tensor("x", [L, E], F32, kind="ExternalInput")
    wq_d = nc.dram_tensor("wq", [CPC, E], F32, kind="ExternalInput")
    wk_d = nc.dram_tensor("wk", [CPC, E], F32, kind="ExternalInput")
    wv_d = nc.dram_tensor("wv", [CPC, E], F32, kind="ExternalInput")
    wo_d = nc.dram_tensor("wo", [E, CPC], F32, kind="ExternalInput")
    lam_d = {
        n: nc.dram_tensor(n, [HALF], F32, kind="ExternalInput")
        for n in ("lq1", "lk1", "lq2", "lk2")
    }
    a_d = nc.dram_tensor("A_out", [HPC, L, L], F32, kind="ExternalOutput")
    o_d = nc.dram_tensor("O_out", [L, E], F32, kind="ExternalOutput")

    with tile.TileContext(nc) as tc, ExitStack() as ctx:
        # ---------------- persistent tiles ----------------
        persist = ctx.enter_context(tc.tile_pool(name="persist", bufs=1))
        QT = persist.tile([128, 2, L], BF16)    # [4heads*32d, qk-tile, t]
        KT = persist.tile([128, 2, L], BF16)
        V = persist.tile([128, 16, CPC], BF16)  # [t-in-block, t-block, c]
        WoT = persist.tile([64, HPC, E], BF16)  # [c-in-pair, pair, e_out]
        OTf = persist.tile([64, HPC, L], BF16)  # normed (A@V).T per pair
        ident = persist.tile([128, 128], F32)
        make_identity(nc, ident)
        identb = persist.tile([128, 128], BF16)
        make_identity(nc, identb)
        # causal mask for the diagonal 128x128 block: 0 on/below diag, -1e30 above
        mdiag = persist.tile([128, 128], F32)
        nc.gpsimd.memset(mdiag, 0.0)
        nc.gpsimd.affine_select(
            out=mdiag, in_=mdiag, compare_op=OP.is_ge, fill=-1e30,
            base=0, channel_multiplier=1, pattern=[[-1, 128]],
        )
        ones64 = persist.tile([64, 1], BF16)
        nc.vector.memset(ones64, 1.0 / 64.0)  # folds the mean(x^2) divisor
        OTsb = persist.tile([64, HPC, L], BF16)  # raw (A@V).T staging for rms
        c3row = persist.tile([1, 64], F32)
        nc.vector.memset(c3row, 1.0 - LAMBDA_INIT)
        epsv = persist.tile([1, 1], F32)
        nc.vector.memset(epsv, 1e-5)
        lam_bc = persist.tile([128, 2], F32)    # col0 = lambda, col1 = 1/lambda

        # ---------------- lambda_full ----------------
        sc = ctx.enter_context(tc.tile_pool(name="lamscal", bufs=1))
        lv = sc.tile([1, 2, 2, HALF], F32)
        for i, n in enumerate(("lq1", "lk1", "lq2", "lk2")):
            nc.sync.dma_start(
                out=lv[:, i // 2, i % 2, :],
                in_=lam_d[n].ap().rearrange("(o a) -> o a", o=1),
            )
        prod = sc.tile([1, 2, HALF], F32)
        nc.vector.tensor_mul(prod, lv[:, :, 0, :], lv[:, :, 1, :])
        dots = sc.tile([1, 2], F32)
        nc.vector.reduce_sum(dots, prod, axis=mybir.AxisListType.X)
        exps = sc.tile([1, 2], F32)
        nc.scalar.activation(exps, dots, AF.Exp)
        lamv = sc.tile([1, 2], F32)
        nc.vector.scalar_tensor_tensor(
            out=lamv[:, 0:1], in0=exps[:, 0:1], scalar=1.0, in1=exps[:, 1:2],
            op0=OP.mult, op1=OP.subtract,
        )
        nc.vector.tensor_scalar_add(lamv[:, 0:1], lamv[:, 0:1], LAMBDA_INIT)
        nc.vector.reciprocal(lamv[:, 1:2], lamv[:, 0:1])
        nc.gpsimd.partition_broadcast(lam_bc, lamv)
        lam_ap = lam_bc[:, 0:1]
        laminv_ap = lam_bc[:, 1:2]

        # ---------------- phase 0/1: transposes + projections ----------------
        with (
            tc.tile_pool(name="ph1sb", bufs=1) as ph1,
            tc.tile_pool(name="ld", bufs=3) as ldp,
            tc.tile_pool(name="trpsum", bufs=2, space="PSUM") as trp,
            tc.tile_pool(name="prjpsum", bufs=2, space="PSUM") as prp,
        ):
            xT = ph1.tile([128, 8, L], BF16)        # [e-in-block, e-block, t]
            WqT = ph1.tile([128, 8, CPC], BF16)     # [e-in-block, e-block, c]
            WkT = ph1.tile([128, 8, CPC], BF16)
            WvT = ph1.tile([128, 8, CPC], BF16)

            # x.T (cast to bf16, then transpose via normal-mode matmul vs I)
            for tb in range(16):
                xt = ldp.tile([128, E], F32, tag="xt")
                nc.sync.dma_start(out=xt, in_=x_d[tb * 128:(tb + 1) * 128, :])
                xtb = ldp.tile([128, E], BF16, tag="xtb")
                nc.any.tensor_copy(xtb, xt)
                tp = trp.tile([128, 8, 128], BF16, tag="tp")
                for e in range(8):
                    nc.tensor.transpose(
                        tp[:, e, :], xtb[:, e * 128:(e + 1) * 128], identb
                    )
                nc.any.tensor_copy(xT[:, :, tb * 128:(tb + 1) * 128], tp)
            # Wq/Wk/Wv transposed
            for wd, wT in ((wq_d, WqT), (wk_d, WkT), (wv_d, WvT)):
                for ct in range(2):
                    wt = ldp.tile([128, E], F32, tag="xt")
                    nc.sync.dma_start(out=wt, in_=wd[ct * 128:(ct + 1) * 128, :])
                    wtb = ldp.tile([128, E], BF16, tag="xtb")
                    nc.any.tensor_copy(wtb, wt)
                    tp = trp.tile([128, 8, 128], BF16, tag="tp")
                    for e in range(8):
                        nc.tensor.transpose(
                            tp[:, e, :], wtb[:, e * 128:(e + 1) * 128], identb
                        )
                    nc.any.tensor_copy(wT[:, :, ct * 128:(ct + 1) * 128], tp)
            # Wo transposed into per-pair [64, E] slabs
            for et in range(8):
                wt = ldp.tile([128, CPC], F32, tag="wo")
                nc.sync.dma_start(out=wt, in_=wo_d[et * 128:(et + 1) * 128, :])
                wtb = ldp.tile([128, CPC], BF16, tag="wob")
                nc.any.tensor_copy(wtb, wt)
                tp = trp.tile([64, 4, 128], BF16, tag="tpo")
                for p in range(HPC):
                    nc.tensor.transpose(
                        tp[:, p, :], wtb[:, p * 64:(p + 1) * 64], identb
                    )
                nc.any.tensor_copy(WoT[:, :, et * 128:(et + 1) * 128], tp)

            # projections: QT/KT (channel-on-partition), V (token-on-partition)
            for ct in range(2):
                for tcn in range(4):
                    for wT, dstT in ((WqT, QT), (WkT, KT)):
                        ps = prp.tile([128, 512], F32, tag="ps")
                        for e in range(8):
                            nc.tensor.matmul(
                                ps,
                                wT[:, e, ct * 128:(ct + 1) * 128],
                                xT[:, e, tcn * 512:(tcn + 1) * 512],
                                start=(e == 0), stop=(e == 7),
                            )
                        nc.any.tensor_copy(
                            dstT[:, ct, tcn * 512:(tcn + 1) * 512], ps
                        )
            for tb in range(16):
                ps = prp.tile([128, CPC], F32, tag="ps")
                for e in range(8):
                    nc.tensor.matmul(
                        ps,
                        xT[:, e, tb * 128:(tb + 1) * 128],
                        WvT[:, e, :],
                        start=(e == 0), stop=(e == 7),
                    )
                nc.any.tensor_copy(V[:, tb, :], ps)

        # ---------------- phase 2: attention ----------------
        ctx2 = ctx.enter_context(ExitStack())
        epool = ctx2.enter_context(tc.tile_pool(name="epool", bufs=2))
        t1pool = ctx2.enter_context(tc.tile_pool(name="t1pool", bufs=2))
        apool = ctx2.enter_context(tc.tile_pool(name="apool", bufs=2))
        abpool = ctx2.enter_context(tc.tile_pool(name="abpool", bufs=2))
        atpool = ctx2.enter_context(tc.tile_pool(name="atpool", bufs=2))
        dpool = ctx2.enter_context(tc.tile_pool(name="dpool", bufs=3))
        spool = ctx2.enter_context(tc.tile_pool(name="spool", bufs=3))
        rmssb = ctx2.enter_context(tc.tile_pool(name="rmssb", bufs=3))
        spsum = ctx2.enter_context(tc.tile_pool(name="spsum", bufs=1, space="PSUM"))
        tpsum = ctx2.enter_context(tc.tile_pool(name="tpsum", bufs=1, space="PSUM"))
        avpsum = ctx2.enter_context(tc.tile_pool(name="avpsum", bufs=1, space="PSUM"))
        smpsum = ctx2.enter_context(tc.tile_pool(name="smpsum", bufs=1, space="PSUM"))

        for pair in range(HPC):
            tpi = pair // 2
            b1 = (pair % 2) * 64
            b2 = b1 + 32
            att = None
            for qb in range(16):
                lk = (qb + 1) * 128
                e1 = epool.tile([128, L], F32, tag="E1")
                e2 = epool.tile([128, L], F32, tag="E2")
                db = dpool.tile([128, 8], F32, tag="db")
                nkc = (lk + 1023) // 1024
                for kc in range(nkc):
                    w = min(1024, lk - kc * 1024)
                    s1 = spsum.tile([128, 1024], F32, tag="S1")
                    s2 = spsum.tile([128, 1024], F32, tag="S2")
                    for s, b in ((s1, b1), (s2, b2)):
                        for half in range(0, w, 512):
                            hw = min(512, w - half)
                            nc.tensor.matmul(
                                s[:, half:half + hw],
                                QT[b:b + 32, tpi, qb * 128:(qb + 1) * 128],
                                KT[b:b + 32, tpi,
                                   kc * 1024 + half:kc * 1024 + half + hw],
                                start=True, stop=True, tile_position=(b, 0),
                            )
                    if kc == nkc - 1:
                        nc.vector.tensor_add(s1[:, w - 128:w], s1[:, w - 128:w], mdiag)
                        nc.vector.tensor_add(s2[:, w - 128:w], s2[:, w - 128:w], mdiag)
                    nc.scalar.activation(
                        e1[:, kc * 1024:kc * 1024 + w], s1[:, :w], AF.Exp,
                        scale=SCALE, accum_out=db[:, kc:kc + 1],
                    )
                    nc.scalar.activation(
                        e2[:, kc * 1024:kc * 1024 + w], s2[:, :w], AF.Exp,
                        scale=SCALE, accum_out=db[:, 4 + kc:4 + kc + 1],
                    )
                ds = spool.tile([128, 2], F32, tag="ds")
                nc.vector.reduce_sum(
                    ds,
                    db.rearrange("p (h k) -> p h k", h=2)[:, :, 0:nkc],
                    axis=mybir.AxisListType.X,
                )
                rs = spool.tile([128, 2], F32, tag="rs")
                nc.vector.reciprocal(rs, ds)
                gf = spool.tile([128, 2], F32, tag="gf")  # col0=g, col1=f2
                nc.vector.scalar_tensor_tensor(
                    out=gf[:, 0:1], in0=ds[:, 1:2], scalar=laminv_ap,
                    in1=rs[:, 0:1], op0=OP.mult, op1=OP.mult,
                )
                nc.vector.tensor_scalar_mul(gf[:, 1:2], rs[:, 1:2], lam_ap)
                t1 = t1pool.tile([128, L], F32, tag="T1")
                nc.vector.scalar_tensor_tensor(
                    out=t1[:, :lk], in0=e1[:, :lk], scalar=gf[:, 0:1],
                    in1=e2[:, :lk], op0=OP.mult, op1=OP.subtract,
                )
                aw = apool.tile([128, L], F32, tag="A")
                nc.vector.tensor_scalar_mul(aw[:, :lk], t1[:, :lk], gf[:, 1:2])
                nc.sync.dma_start(
                    out=a_d[pair, qb * 128:(qb + 1) * 128, 0:lk], in_=aw[:, :lk]
                )
                # bf16 copy of A (scaled) feeding the transposes
                ab = abpool.tile([128, L], BF16, tag="Ab")
                nc.vector.tensor_scalar_mul(ab[:, :lk], t1[:, :lk], gf[:, 1:2])
                # transpose A for the A@V matmul (normal-mode matmul vs I)
                if qb % 4 == 0:
                    att = atpool.tile([128, 16, 512], BF16, tag="AT")
                qc = (qb % 4) * 128
                for g8 in range((qb + 8) // 8):
                    nb = min(8, qb + 1 - g8 * 8)
                    tp = tpsum.tile([128, 8, 128], BF16, tag="tp")
                    for j in range(nb):
                        kb = g8 * 8 + j
                        nc.tensor.transpose(
                            tp[:, j, :], ab[:, kb * 128:(kb + 1) * 128], identb
                        )
                    nc.any.tensor_copy(
                        att[:, g8 * 8:g8 * 8 + nb, qc:qc + 128], tp[:, 0:nb, :]
                    )
                if qb % 4 == 3:
                    qg = qb // 4
                    ot = avpsum.tile([64, 512], F32, tag="ot")
                    nkb = (qg + 1) * 4
                    for kb in range(nkb):
                        c0 = max(0, kb * 128 - qg * 512)
                        nc.tensor.matmul(
                            ot[:, c0:512],
                            V[:, kb, pair * 64:(pair + 1) * 64],
                            att[:, kb, c0:512],
                            start=(kb == 0), stop=(kb == nkb - 1),
                        )
                    nc.any.tensor_copy(
                        OTsb[:, pair, qg * 512:(qg + 1) * 512], ot
                    )

        # ---------------- phase 2.5: batched rmsnorm over head dim ----------
        tc.no_sync_barrier()
        for pair in range(HPC):
            for qg in range(4):
                osl = OTsb[:, pair, qg * 512:(qg + 1) * 512]
                sq = rmssb.tile([64, 512], BF16, tag="sq")
                nc.vector.tensor_mul(sq, osl, osl)
                ms = smpsum.tile([1, 512], F32, tag="sm")
                nc.tensor.matmul(ms, ones64, sq, start=True, stop=True)
                rt = rmssb.tile([1, 512], F32, tag="rt")
                nc.scalar.activation(rt, ms, AF.Sqrt, bias=epsv)
                inv = rmssb.tile([1, 512], F32, tag="inv")
                scr = rmssb.tile([1, 512], F32, tag="scr")
                nc.vector.reciprocal_approx_accurate(inv, rt, scr)
                bc = smpsum.tile([64, 512], F32, tag="sm")
                nc.tensor.matmul(bc, c3row, inv, start=True, stop=True)
                nc.vector.tensor_mul(
                    OTf[:, pair, qg * 512:(qg + 1) * 512], osl, bc
                )

        # ---------------- phase 3: o_proj (partial) ----------------
        ctx2.close()
        with (
            tc.tile_pool(name="oout", bufs=3) as oop,
            tc.tile_pool(name="opsum", bufs=2, space="PSUM") as ops,
        ):
            for eo in range(2):
                for tb in range(16):
                    po = ops.tile([128, 512], F32, tag="po")
                    for p in range(HPC):
                        nc.tensor.matmul(
                            po,
                            OTf[:, p, tb * 128:(tb + 1) * 128],
                            WoT[:, p, eo * 512:(eo + 1) * 512],
                            start=(p == 0), stop=(p == 3),
                        )
                    ob = oop.tile([128, 512], F32, tag="ob")
                    nc.any.tensor_copy(ob, po)
                    nc.sync.dma_start(
                        out=o_d[tb * 128:(tb + 1) * 128, eo * 512:(eo + 1) * 512],
                        in_=ob,
                    )

    nc.finalize()
    _CACHE["nc"] = nc
    return nc


def kernel(x, Wq, Wk, Wv, Wo, lambda_q1, lambda_k1, lambda_q2, lambda_k2):
    global LAST_EXEC_NS
    x = np.asarray(x, np.float32)
    Wq = np.asarray(Wq, np.float32)
    Wk = np.asarray(Wk, np.float32)
    Wv = np.asarray(Wv, np.float32)
    Wo = np.asarray(Wo, np.float32)

    nc = _build()
    in_maps = []
    for c in range(N_CORES):
        b, g = divmod(c, HPC)
        s = slice(g * CPC, (g + 1) * CPC)
        in_maps.append({
            "x": np.ascontiguousarray(x[b]),
            "wq": np.ascontiguousarray(Wq[s, :]),
            "wk": np.ascontiguousarray(Wk[s, :]),
            "wv": np.ascontiguousarray(Wv[s, :]),
            "wo": np.ascontiguousarray(Wo[:, s]),
            "lq1": np.asarray(lambda_q1, np.float32),
            "lk1": np.asarray(lambda_k1, np.float32),
            "lq2": np.asarray(lambda_q2, np.float32),
            "lk2": np.asarray(lambda_k2, np.float32),
        })

    res = run_bass_kernel_spmd(
        nc, in_maps, core_ids=list(range(N_CORES)), trace=TRACE
    )
    LAST_EXEC_NS = res.exec_time_ns

    A = np.empty((B, NH, L, L), np.float32)
    O = np.zeros((B, L, E), np.float32)
    for c in range(N_CORES):
        b, g = divmod(c, HPC)
        A[b, g * HPC:(g + 1) * HPC] = res.results[c]["A_out"]
        O[b] += res.results[c]["O_out"]
    return O, A


# revision 25
# speedup vs baseline: 1.1344x; 1.1344x over previous
"""Trainium2 Bass kernel for causal multi-head differential attention.

Reference semantics (per batch b):
  Q = x @ Wq.T -> [L, 2*NH, 32], K likewise, V = x @ Wv.T -> [L, NH, 64]
  scores = Q K^T / sqrt(32), causal-masked, softmax
  lambda_full = exp(lq1.lk1) - exp(lq2.lk2) + 0.2
  A = p_even - lambda_full * p_odd            (per V-head)
  O = rmsnorm(A @ V) * (1 - 0.2);  out = O @ Wo.T
Returns (out [B,L,E], A [B,NH,L,L]).

Sharding: 8 cores = 2 batches x 4 head-groups. Each core owns one batch and
4 V-heads (8 paired Q/K heads), computes its A shard and a partial o_proj
output; the host concatenates A shards and sums the 4 o_proj partials per
batch (tensor-parallel unshard).
"""

import math
import sys

import numpy as np


def _ensure_paths():
    try:
        import concourse.bass  # noqa: F401
        return
    except ImportError:
        pass
    for p in (
        "/root/.axon_site",
        "/root/.axon_site/_ro/trn_rl_repo",
        "/root/.axon_site/_ro/pypackages",
        "/opt/trn_rl_repo",
    ):
        if p not in sys.path:
            sys.path.append(p)


_ensure_paths()

from contextlib import ExitStack

import concourse.bass as bass  # noqa: E402
import concourse.tile as tile  # noqa: E402
from concourse import bacc, mybir  # noqa: E402
from concourse.bass_utils import run_bass_kernel_spmd  # noqa: E402
from concourse.masks import make_identity  # noqa: E402

B, L, E = 2, 2048, 1024
NH = 16
HD = 64
HALF = 32
LAMBDA_INIT = 0.2
SCALE = 1.0 / math.sqrt(HALF)
N_CORES = 8
HPC = 4          # V-heads per core
CPC = HPC * HD   # channels per core (256)

F32 = mybir.dt.float32
BF16 = mybir.dt.bfloat16
AF = mybir.ActivationFunctionType
OP = mybir.AluOpType

TRACE = False
LAST_EXEC_NS = None

_CACHE = {}


def _build():
    if "nc" in _CACHE:
        return _CACHE["nc"]

    nc = bacc.Bacc(None)

    x_d = nc.dram_tensor("x", [L, E], F32, kind="ExternalInput")
    wq_d = nc.dram_tensor("wq", [CPC, E], F32, kind="ExternalInput")
    wk_d = nc.dram_tensor("wk", [CPC, E], F32, kind="ExternalInput")
    wv_d = nc.dram_tensor("wv", [CPC, E], F32, kind="ExternalInput")
    wo_d = nc.dram_tensor("wo", [E, CPC], F32, kind="ExternalInput")
    lam_d = {
        n: nc.dram_tensor(n, [HALF], F32, kind="ExternalInput")
        for n in ("lq1", "lk1", "lq2", "lk2")
    }
    a_d = nc.dram_tensor("A_out", [HPC, L, L], F32, kind="ExternalOutput")
    o_d = nc.dram_tensor("O_out", [L, E], F32, kind="ExternalOutput")

    with tile.TileContext(nc) as tc, ExitStack() as ctx:
        # ---------------- persistent tiles ----------------
        persist = ctx.enter_context(tc.tile_pool(name="persist", bufs=1))
        QT = persist.tile([128, 2, L], BF16)    # [4heads*32d, qk-tile, t]
        KT = persist.tile([128, 2, L], BF16)
        V = persist.tile([128, 16, CPC], BF16)  # [t-in-block, t-block, c]
        WoT = persist.tile([64, HPC, E], BF16)  # [c-in-pair, pair, e_out]
        OTf = persist.tile([64, HPC, L], BF16)  # normed (A@V).T per pair
        ident = persist.tile([128, 128], F32)
        make_identity(nc, ident)
        identb = persist.tile([128, 128], BF16)
        make_identity(nc, identb)
        # transposed causal mask for the diagonal block, applied by PE as an
        # accumulating matmul: S[q, k] += mdiagT[k, q] (0 on/below diag, -1e30
        # above). mdiagT[k, q] = 0 where q >= k.
        mdiagT = persist.tile([128, 128], BF16)
        nc.gpsimd.memset(mdiagT, 0.0)
        nc.gpsimd.affine_select(
            out=mdiagT, in_=mdiagT, compare_op=OP.is_ge, fill=-1e30,
            base=0, channel_multiplier=-1, pattern=[[1, 128]],
        )
        ones64 = persist.tile([64, 1], BF16)
        nc.vector.memset(ones64, 1.0 / 64.0)  # folds the mean(x^2) divisor
        OTsb = persist.tile([64, HPC, L], BF16)  # raw (A@V).T staging for rms
        c3row = persist.tile([1, 64], F32)
        nc.vector.memset(c3row, 1.0 - LAMBDA_INIT)
        epsv = persist.tile([1, 1], F32)
        nc.vector.memset(epsv, 1e-5)
        msall = persist.tile([16, 512], F32)     # per-(pair,qgroup) mean-sq rows
        eps16 = persist.tile([16, 1], F32)
        nc.vector.memset(eps16, 1e-5)
        # c3sel[:, i*64:(i+1)*64] is a [16, 64] selector: 0.8 on row i, else 0.
        # bc_i = c3sel_i.T @ invall broadcasts inv row i to 64 partitions.
        c3sel = persist.tile([16, 1024], F32)
        nc.gpsimd.memset(c3sel, 1.0 - LAMBDA_INIT)
        nc.gpsimd.affine_select(
            out=c3sel, in_=c3sel, compare_op=OP.is_ge, fill=0.0,
            base=0, channel_multiplier=-64, pattern=[[1, 1024]],
        )
        nc.gpsimd.affine_select(
            out=c3sel, in_=c3sel, compare_op=OP.is_ge, fill=0.0,
            base=63, channel_multiplier=64, pattern=[[-1, 1024]],
        )
        lam_bc = persist.tile([128, 2], F32)    # col0 = lambda, col1 = 1/lambda

        # ---------------- lambda_full ----------------
        sc = ctx.enter_context(tc.tile_pool(name="lamscal", bufs=1))
        lv = sc.tile([1, 2, 2, HALF], F32)
        for i, n in enumerate(("lq1", "lk1", "lq2", "lk2")):
            nc.sync.dma_start(
                out=lv[:, i // 2, i % 2, :],
                in_=lam_d[n].ap().rearrange("(o a) -> o a", o=1),
            )
        prod = sc.tile([1, 2, HALF], F32)
        nc.vector.tensor_mul(prod, lv[:, :, 0, :], lv[:, :, 1, :])
        dots = sc.tile([1, 2], F32)
        nc.vector.reduce_sum(dots, prod, axis=mybir.AxisListType.X)
        exps = sc.tile([1, 2], F32)
        nc.scalar.activation(exps, dots, AF.Exp)
        lamv = sc.tile([1, 2], F32)
        nc.vector.scalar_tensor_tensor(
            out=lamv[:, 0:1], in0=exps[:, 0:1], scalar=1.0, in1=exps[:, 1:2],
            op0=OP.mult, op1=OP.subtract,
        )
        nc.vector.tensor_scalar_add(lamv[:, 0:1], lamv[:, 0:1], LAMBDA_INIT)
        nc.vector.reciprocal(lamv[:, 1:2], lamv[:, 0:1])
        nc.gpsimd.partition_broadcast(lam_bc, lamv)
        lam_ap = lam_bc[:, 0:1]
        laminv_ap = lam_bc[:, 1:2]

        # ---------------- phase 0/1: transposes + projections ----------------
        with (
            tc.tile_pool(name="ph1sb", bufs=1) as ph1,
            tc.tile_pool(name="ld", bufs=3) as ldp,
            tc.tile_pool(name="trpsum", bufs=2, space="PSUM") as trp,
            tc.tile_pool(name="prjpsum", bufs=2, space="PSUM") as prp,
        ):
            xT = ph1.tile([128, 8, L], BF16)        # [e-in-block, e-block, t]
            WqT = ph1.tile([128, 8, CPC], BF16)     # [e-in-block, e-block, c]
            WkT = ph1.tile([128, 8, CPC], BF16)
            WvT = ph1.tile([128, 8, CPC], BF16)

            # x.T (cast to bf16, then transpose via normal-mode matmul vs I)
            for tb in range(16):
                xt = ldp.tile([128, E], F32, tag="xt")
                nc.sync.dma_start(out=xt, in_=x_d[tb * 128:(tb + 1) * 128, :])
                xtb = ldp.tile([128, E], BF16, tag="xtb")
                nc.any.tensor_copy(xtb, xt)
                tp = trp.tile([128, 8, 128], BF16, tag="tp")
                for e in range(8):
                    nc.tensor.transpose(
                        tp[:, e, :], xtb[:, e * 128:(e + 1) * 128], identb
                    )
                nc.any.tensor_copy(xT[:, :, tb * 128:(tb + 1) * 128], tp)
            # Wq/Wk/Wv transposed
            for wd, wT in ((wq_d, WqT), (wk_d, WkT), (wv_d, WvT)):
                for ct in range(2):
                    wt = ldp.tile([128, E], F32, tag="xt")
                    nc.sync.dma_start(out=wt, in_=wd[ct * 128:(ct + 1) * 128, :])
                    wtb = ldp.tile([128, E], BF16, tag="xtb")
                    nc.any.tensor_copy(wtb, wt)
                    tp = trp.tile([128, 8, 128], BF16, tag="tp")
                    for e in range(8):
                        nc.tensor.transpose(
                            tp[:, e, :], wtb[:, e * 128:(e + 1) * 128], identb
                        )
                    nc.any.tensor_copy(wT[:, :, ct * 128:(ct + 1) * 128], tp)
            # Wo transposed into per-pair [64, E] slabs
            for et in range(8):
                wt = ldp.tile([128, CPC], F32, tag="wo")
                nc.sync.dma_start(out=wt, in_=wo_d[et * 128:(et + 1) * 128, :])
                wtb = ldp.tile([128, CPC], BF16, tag="wob")
                nc.any.tensor_copy(wtb, wt)
                tp = trp.tile([64, 4, 128], BF16, tag="tpo")
                for p in range(HPC):
                    nc.tensor.transpose(
                        tp[:, p, :], wtb[:, p * 64:(p + 1) * 64], identb
                    )
                nc.any.tensor_copy(WoT[:, :, et * 128:(et + 1) * 128], tp)

            # projections: QT/KT (channel-on-partition), V (token-on-partition)
            for ct in range(2):
                for tcn in range(4):
                    for wT, dstT in ((WqT, QT), (WkT, KT)):
                        ps = prp.tile([128, 512], F32, tag="ps")
                        for e in range(8):
                            nc.tensor.matmul(
                                ps,
                                wT[:, e, ct * 128:(ct + 1) * 128],
                                xT[:, e, tcn * 512:(tcn + 1) * 512],
                                start=(e == 0), stop=(e == 7),
                            )
                        nc.any.tensor_copy(
                            dstT[:, ct, tcn * 512:(tcn + 1) * 512], ps
                        )
            for tb in range(16):
                ps = prp.tile([128, CPC], F32, tag="ps")
                for e in range(8):
                    nc.tensor.matmul(
                        ps,
                        xT[:, e, tb * 128:(tb + 1) * 128],
                        WvT[:, e, :],
                        start=(e == 0), stop=(e == 7),
                    )
                nc.any.tensor_copy(V[:, tb, :], ps)

        # ---------------- phase 2: attention ----------------
        ctx2 = ctx.enter_context(ExitStack())
        epool = ctx2.enter_context(tc.tile_pool(name="epool", bufs=2))
        t1pool = ctx2.enter_context(tc.tile_pool(name="t1pool", bufs=2))
        apool = ctx2.enter_context(tc.tile_pool(name="apool", bufs=2))
        abpool = ctx2.enter_context(tc.tile_pool(name="abpool", bufs=2))
        atpool = ctx2.enter_context(tc.tile_pool(name="atpool", bufs=2))
        dpool = ctx2.enter_context(tc.tile_pool(name="dpool", bufs=3))
        spool = ctx2.enter_context(tc.tile_pool(name="spool", bufs=3))
        rmssb = ctx2.enter_context(tc.tile_pool(name="rmssb", bufs=3))
        rms1 = ctx2.enter_context(tc.tile_pool(name="rms1", bufs=1))
        spsum = ctx2.enter_context(tc.tile_pool(name="spsum", bufs=2, space="PSUM"))
        tpsum = ctx2.enter_context(tc.tile_pool(name="tpsum", bufs=1, space="PSUM"))
        avpsum = ctx2.enter_context(tc.tile_pool(name="avpsum", bufs=1, space="PSUM"))
        smpsum = ctx2.enter_context(tc.tile_pool(name="smpsum", bufs=1, space="PSUM"))

        for pair in range(HPC):
            tpi = pair // 2
            b1 = (pair % 2) * 64
            b2 = b1 + 32
            att = None
            for qb in range(16):
                lk = (qb + 1) * 128
                e1 = epool.tile([128, L], F32, tag="E1")
                e2 = epool.tile([128, L], F32, tag="E2")
                db = dpool.tile([128, 8], F32, tag="db")
                nkc = (lk + 511) // 512
                for kc in range(nkc):
                    w = min(512, lk - kc * 512)
                    s1 = spsum.tile([128, 512], F32, tag="S1")
                    s2 = spsum.tile([128, 512], F32, tag="S2")
                    for s, b in ((s1, b1), (s2, b2)):
                        nc.tensor.matmul(
                            s[:, :w],
                            QT[b:b + 32, tpi, qb * 128:(qb + 1) * 128],
                            KT[b:b + 32, tpi, kc * 512:kc * 512 + w],
                            start=True, stop=True, tile_position=(b, 0),
                        )
                        if kc == nkc - 1:
                            nc.tensor.matmul(
                                s[:, w - 128:w], mdiagT, identb,
                                start=False, stop=True,
                            )
                    nc.scalar.activation(
                        e1[:, kc * 512:kc * 512 + w], s1[:, :w], AF.Exp,
                        scale=SCALE, accum_out=db[:, kc:kc + 1],
                    )
                    nc.scalar.activation(
                        e2[:, kc * 512:kc * 512 + w], s2[:, :w], AF.Exp,
                        scale=SCALE, accum_out=db[:, 4 + kc:4 + kc + 1],
                    )
                ds = spool.tile([128, 2], F32, tag="ds")
                nc.vector.reduce_sum(
                    ds,
                    db.rearrange("p (h k) -> p h k", h=2)[:, :, 0:nkc],
                    axis=mybir.AxisListType.X,
                )
                rs = spool.tile([128, 2], F32, tag="rs")
                nc.vector.reciprocal(rs, ds)
                gf = spool.tile([128, 2], F32, tag="gf")  # col0=g, col1=f2
                nc.vector.scalar_tensor_tensor(
                    out=gf[:, 0:1], in0=ds[:, 1:2], scalar=laminv_ap,
                    in1=rs[:, 0:1], op0=OP.mult, op1=OP.mult,
                )
                nc.vector.tensor_scalar_mul(gf[:, 1:2], rs[:, 1:2], lam_ap)
                t1 = t1pool.tile([128, L], F32, tag="T1")
                nc.vector.scalar_tensor_tensor(
                    out=t1[:, :lk], in0=e1[:, :lk], scalar=gf[:, 0:1],
                    in1=e2[:, :lk], op0=OP.mult, op1=OP.subtract,
                )
                aw = apool.tile([128, L], F32, tag="A")
                nc.vector.tensor_scalar_mul(aw[:, :lk], t1[:, :lk], gf[:, 1:2])
                nc.sync.dma_start(
                    out=a_d[pair, qb * 128:(qb + 1) * 128, 0:lk], in_=aw[:, :lk]
                )
                # bf16 copy of A (scaled) feeding the transposes
                ab = abpool.tile([128, L], BF16, tag="Ab")
                nc.vector.tensor_scalar_mul(ab[:, :lk], t1[:, :lk], gf[:, 1:2])
                # transpose A for the A@V matmul (normal-mode matmul vs I)
                if qb % 4 == 0:
                    att = atpool.tile([128, 16, 512], BF16, tag="AT")
                qc = (qb % 4) * 128
                for g8 in range((qb + 8) // 8):
                    nb = min(8, qb + 1 - g8 * 8)
                    tp = tpsum.tile([128, 8, 128], BF16, tag="tp")
                    for j in range(nb):
                        kb = g8 * 8 + j
                        nc.tensor.transpose(
                            tp[:, j, :], ab[:, kb * 128:(kb + 1) * 128], identb
                        )
                    nc.any.tensor_copy(
                        att[:, g8 * 8:g8 * 8 + nb, qc:qc + 128], tp[:, 0:nb, :]
                    )
                if qb % 4 == 3:
                    qg = qb // 4
                    ot = avpsum.tile([64, 512], F32, tag="ot")
                    nkb = (qg + 1) * 4
                    for kb in range(nkb):
                        c0 = max(0, kb * 128 - qg * 512)
                        nc.tensor.matmul(
                            ot[:, c0:512],
                            V[:, kb, pair * 64:(pair + 1) * 64],
                            att[:, kb, c0:512],
                            start=(kb == 0), stop=(kb == nkb - 1),
                        )
                    nc.any.tensor_copy(
                        OTsb[:, pair, qg * 512:(qg + 1) * 512], ot
                    )

        # ---------------- phase 2.5: batched rmsnorm over head dim ----------
        for pair in range(HPC):
            for qg in range(4):
                osl = OTsb[:, pair, qg * 512:(qg + 1) * 512]
                sq = rmssb.tile([64, 512], BF16, tag="sq")
                nc.vector.tensor_mul(sq, osl, osl)
                ms = smpsum.tile([1, 512], F32, tag="sm")
                nc.tensor.matmul(ms, ones64, sq, start=True, stop=True)
                mss = rmssb.tile([1, 512], F32, tag="mss")
                nc.any.tensor_copy(mss, ms)
                nc.sync.dma_start(
                    out=msall[pair * 4 + qg:pair * 4 + qg + 1, :], in_=mss
                )
        rtall = rms1.tile([16, 512], F32, tag="rtall")
        nc.scalar.activation(rtall, msall, AF.Sqrt, bias=eps16)
        invall = rms1.tile([16, 512], F32, tag="invall")
        scrall = rms1.tile([16, 512], F32, tag="scrall")
        nc.vector.reciprocal_approx_accurate(invall, rtall, scrall)
        for pair in range(HPC):
            for qg in range(4):
                i = pair * 4 + qg
                bc = smpsum.tile([64, 512], F32, tag="sm")
                nc.tensor.matmul(
                    bc, c3sel[:, i * 64:(i + 1) * 64], invall,
                    start=True, stop=True,
                )
                nc.vector.tensor_mul(
                    OTf[:, pair, qg * 512:(qg + 1) * 512],
                    OTsb[:, pair, qg * 512:(qg + 1) * 512], bc
                )

        # ---------------- phase 3: o_proj (partial) ----------------
        ctx2.close()
        with (
            tc.tile_pool(name="oout", bufs=3) as oop,
            tc.tile_pool(name="opsum", bufs=2, space="PSUM") as ops,
        ):
            for eo in range(2):
                for tb in range(16):
                    po = ops.tile([128, 512], F32, tag="po")
                    for p in range(HPC):
                        nc.tensor.matmul(
                            po,
                            OTf[:, p, tb * 128:(tb + 1) * 128],
                            WoT[:, p, eo * 512:(eo + 1) * 512],
                            start=(p == 0), stop=(p == 3),
                        )
                    ob = oop.tile([128, 512], F32, tag="ob")
                    nc.any.tensor_copy(ob, po)
                    nc.sync.dma_start(
                        out=o_d[tb * 128:(tb + 1) * 128, eo * 512:(eo + 1) * 512],
                        in_=ob,
                    )

    nc.finalize()
    _CACHE["nc"] = nc
    return nc


def kernel(x, Wq, Wk, Wv, Wo, lambda_q1, lambda_k1, lambda_q2, lambda_k2):
    global LAST_EXEC_NS
    x = np.asarray(x, np.float32)
    Wq = np.asarray(Wq, np.float32)
    Wk = np.asarray(Wk, np.float32)
    Wv = np.asarray(Wv, np.float32)
    Wo = np.asarray(Wo, np.float32)

    nc = _build()
    in_maps = []
    for c in range(N_CORES):
        b, g = divmod(c, HPC)
        s = slice(g * CPC, (g + 1) * CPC)
        in_maps.append({
            "x": np.ascontiguousarray(x[b]),
            "wq": np.ascontiguousarray(Wq[s, :]),
            "wk": np.ascontiguousarray(Wk[s, :]),
            "wv": np.ascontiguousarray(Wv[s, :]),
            "wo": np.ascontiguousarray(Wo[:, s]),
            "lq1": np.asarray(lambda_q1, np.float32),
            "lk1": np.asarray(lambda_k1, np.float32),
            "lq2": np.asarray(lambda_q2, np.float32),
            "lk2": np.asarray(lambda_k2, np.float32),
        })

    res = run_bass_kernel_spmd(
        nc, in_maps, core_ids=list(range(N_CORES)), trace=TRACE
    )
    LAST_EXEC_NS = res.exec_time_ns

    A = np.empty((B, NH, L, L), np.float32)
    O = np.zeros((B, L, E), np.float32)
    for c in range(N_CORES):
        b, g = divmod(c, HPC)
        A[b, g * HPC:(g + 1) * HPC] = res.results[c]["A_out"]
        O[b] += res.results[c]["O_out"]
    return O, A


# revision 26
# speedup vs baseline: 1.1478x; 1.0119x over previous
"""Trainium2 Bass kernel for causal multi-head differential attention.

Reference semantics (per batch b):
  Q = x @ Wq.T -> [L, 2*NH, 32], K likewise, V = x @ Wv.T -> [L, NH, 64]
  scores = Q K^T / sqrt(32), causal-masked, softmax
  lambda_full = exp(lq1.lk1) - exp(lq2.lk2) + 0.2
  A = p_even - lambda_full * p_odd            (per V-head)
  O = rmsnorm(A @ V) * (1 - 0.2);  out = O @ Wo.T
Returns (out [B,L,E], A [B,NH,L,L]).

Sharding: 8 cores = 2 batches x 4 head-groups. Each core owns one batch and
4 V-heads (8 paired Q/K heads), computes its A shard and a partial o_proj
output; the host concatenates A shards and sums the 4 o_proj partials per
batch (tensor-parallel unshard).
"""

import math
import sys

import numpy as np


def _ensure_paths():
    try:
        import concourse.bass  # noqa: F401
        return
    except ImportError:
        pass
    for p in (
        "/root/.axon_site",
        "/root/.axon_site/_ro/trn_rl_repo",
        "/root/.axon_site/_ro/pypackages",
        "/opt/trn_rl_repo",
    ):
        if p not in sys.path:
            sys.path.append(p)


_ensure_paths()

from contextlib import ExitStack

import concourse.bass as bass  # noqa: E402
import concourse.tile as tile  # noqa: E402
from concourse import bacc, mybir  # noqa: E402
from concourse.bass_utils import run_bass_kernel_spmd  # noqa: E402
from concourse.masks import make_identity  # noqa: E402

B, L, E = 2, 2048, 1024
NH = 16
HD = 64
HALF = 32
LAMBDA_INIT = 0.2
SCALE = 1.0 / math.sqrt(HALF)
N_CORES = 8
HPC = 4          # V-heads per core
CPC = HPC * HD   # channels per core (256)

F32 = mybir.dt.float32
BF16 = mybir.dt.bfloat16
AF = mybir.ActivationFunctionType
OP = mybir.AluOpType

TRACE = False
LAST_EXEC_NS = None

_CACHE = {}


def _build():
    if "nc" in _CACHE:
        return _CACHE["nc"]

    nc = bacc.Bacc(None)

    x_d = nc.dram_tensor("x", [L, E], F32, kind="ExternalInput")
    wq_d = nc.dram_tensor("wq", [CPC, E], F32, kind="ExternalInput")
    wk_d = nc.dram_tensor("wk", [CPC, E], F32, kind="ExternalInput")
    wv_d = nc.dram_tensor("wv", [CPC, E], F32, kind="ExternalInput")
    wo_d = nc.dram_tensor("wo", [E, CPC], F32, kind="ExternalInput")
    lam_d = {
        n: nc.dram_tensor(n, [HALF], F32, kind="ExternalInput")
        for n in ("lq1", "lk1", "lq2", "lk2")
    }
    a_d = nc.dram_tensor("A_out", [HPC, L, L], F32, kind="ExternalOutput")
    o_d = nc.dram_tensor("O_out", [L, E], F32, kind="ExternalOutput")

    with tile.TileContext(nc) as tc, ExitStack() as ctx:
        # ---------------- persistent tiles ----------------
        persist = ctx.enter_context(tc.tile_pool(name="persist", bufs=1))
        QT = persist.tile([128, 2, L], BF16)    # [4heads*32d, qk-tile, t]
        KT = persist.tile([128, 2, L], BF16)
        V = persist.tile([128, 16, CPC], BF16)  # [t-in-block, t-block, c]
        WoT = persist.tile([64, HPC, E], BF16)  # [c-in-pair, pair, e_out]
        OTf = persist.tile([64, HPC, L], BF16)  # normed (A@V).T per pair
        ident = persist.tile([128, 128], F32)
        make_identity(nc, ident)
        identb = persist.tile([128, 128], BF16)
        make_identity(nc, identb)
        # transposed causal mask for the diagonal block, applied by PE as an
        # accumulating matmul: S[q, k] += mdiagT[k, q] (0 on/below diag, -1e30
        # above). mdiagT[k, q] = 0 where q >= k.
        mdiagT = persist.tile([128, 128], BF16)
        nc.gpsimd.memset(mdiagT, 0.0)
        nc.gpsimd.affine_select(
            out=mdiagT, in_=mdiagT, compare_op=OP.is_ge, fill=-1e30,
            base=0, channel_multiplier=-1, pattern=[[1, 128]],
        )
        ones64 = persist.tile([64, 1], BF16)
        nc.vector.memset(ones64, 1.0 / 64.0)  # folds the mean(x^2) divisor
        OTsb = persist.tile([64, HPC, L], BF16)  # raw (A@V).T staging for rms
        c3row = persist.tile([1, 64], F32)
        nc.vector.memset(c3row, 1.0 - LAMBDA_INIT)
        ms4 = persist.tile([4, 512], F32)       # per-qgroup mean-sq rows
        eps4 = persist.tile([4, 1], F32)
        nc.vector.memset(eps4, 1e-5)
        # c3sel4[:, p*64:(p+1)*64] is a [4, 64] selector: 0.8 on row p, else 0.
        # bc_p = c3sel4_p.T @ inv4 broadcasts inv row p to 64 partitions.
        c3sel4 = persist.tile([4, 256], F32)
        nc.gpsimd.memset(c3sel4, 1.0 - LAMBDA_INIT)
        nc.gpsimd.affine_select(
            out=c3sel4, in_=c3sel4, compare_op=OP.is_ge, fill=0.0,
            base=0, channel_multiplier=-64, pattern=[[1, 256]],
        )
        nc.gpsimd.affine_select(
            out=c3sel4, in_=c3sel4, compare_op=OP.is_ge, fill=0.0,
            base=63, channel_multiplier=64, pattern=[[-1, 256]],
        )
        lam_bc = persist.tile([128, 2], F32)    # col0 = lambda, col1 = 1/lambda

        # ---------------- lambda_full ----------------
        sc = ctx.enter_context(tc.tile_pool(name="lamscal", bufs=1))
        lv = sc.tile([1, 2, 2, HALF], F32)
        for i, n in enumerate(("lq1", "lk1", "lq2", "lk2")):
            nc.sync.dma_start(
                out=lv[:, i // 2, i % 2, :],
                in_=lam_d[n].ap().rearrange("(o a) -> o a", o=1),
            )
        prod = sc.tile([1, 2, HALF], F32)
        nc.vector.tensor_mul(prod, lv[:, :, 0, :], lv[:, :, 1, :])
        dots = sc.tile([1, 2], F32)
        nc.vector.reduce_sum(dots, prod, axis=mybir.AxisListType.X)
        exps = sc.tile([1, 2], F32)
        nc.scalar.activation(exps, dots, AF.Exp)
        lamv = sc.tile([1, 2], F32)
        nc.vector.scalar_tensor_tensor(
            out=lamv[:, 0:1], in0=exps[:, 0:1], scalar=1.0, in1=exps[:, 1:2],
            op0=OP.mult, op1=OP.subtract,
        )
        nc.vector.tensor_scalar_add(lamv[:, 0:1], lamv[:, 0:1], LAMBDA_INIT)
        nc.vector.reciprocal(lamv[:, 1:2], lamv[:, 0:1])
        nc.gpsimd.partition_broadcast(lam_bc, lamv)
        lam_ap = lam_bc[:, 0:1]
        laminv_ap = lam_bc[:, 1:2]

        # ---------------- phase 0/1: transposes + projections ----------------
        with (
            tc.tile_pool(name="ph1sb", bufs=1) as ph1,
            tc.tile_pool(name="ld", bufs=3) as ldp,
            tc.tile_pool(name="trpsum", bufs=2, space="PSUM") as trp,
            tc.tile_pool(name="prjpsum", bufs=2, space="PSUM") as prp,
        ):
            xT = ph1.tile([128, 8, L], BF16)        # [e-in-block, e-block, t]
            WqT = ph1.tile([128, 8, CPC], BF16)     # [e-in-block, e-block, c]
            WkT = ph1.tile([128, 8, CPC], BF16)
            WvT = ph1.tile([128, 8, CPC], BF16)

            # x.T (cast to bf16, then transpose via normal-mode matmul vs I)
            for tb in range(16):
                xt = ldp.tile([128, E], F32, tag="xt")
                nc.sync.dma_start(out=xt, in_=x_d[tb * 128:(tb + 1) * 128, :])
                xtb = ldp.tile([128, E], BF16, tag="xtb")
                nc.any.tensor_copy(xtb, xt)
                tp = trp.tile([128, 8, 128], BF16, tag="tp")
                for e in range(8):
                    nc.tensor.transpose(
                        tp[:, e, :], xtb[:, e * 128:(e + 1) * 128], identb
                    )
                nc.any.tensor_copy(xT[:, :, tb * 128:(tb + 1) * 128], tp)
            # Wq/Wk/Wv transposed
            for wd, wT in ((wq_d, WqT), (wk_d, WkT), (wv_d, WvT)):
                for ct in range(2):
                    wt = ldp.tile([128, E], F32, tag="xt")
                    nc.sync.dma_start(out=wt, in_=wd[ct * 128:(ct + 1) * 128, :])
                    wtb = ldp.tile([128, E], BF16, tag="xtb")
                    nc.any.tensor_copy(wtb, wt)
                    tp = trp.tile([128, 8, 128], BF16, tag="tp")
                    for e in range(8):
                        nc.tensor.transpose(
                            tp[:, e, :], wtb[:, e * 128:(e + 1) * 128], identb
                        )
                    nc.any.tensor_copy(wT[:, :, ct * 128:(ct + 1) * 128], tp)
            # Wo transposed into per-pair [64, E] slabs
            for et in range(8):
                wt = ldp.tile([128, CPC], F32, tag="wo")
                nc.sync.dma_start(out=wt, in_=wo_d[et * 128:(et + 1) * 128, :])
                wtb = ldp.tile([128, CPC], BF16, tag="wob")
                nc.any.tensor_copy(wtb, wt)
                tp = trp.tile([64, 4, 128], BF16, tag="tpo")
                for p in range(HPC):
                    nc.tensor.transpose(
                        tp[:, p, :], wtb[:, p * 64:(p + 1) * 64], identb
                    )
                nc.any.tensor_copy(WoT[:, :, et * 128:(et + 1) * 128], tp)

            # projections: QT/KT (channel-on-partition), V (token-on-partition)
            for ct in range(2):
                for tcn in range(4):
                    for wT, dstT in ((WqT, QT), (WkT, KT)):
                        ps = prp.tile([128, 512], F32, tag="ps")
                        for e in range(8):
                            nc.tensor.matmul(
                                ps,
                                wT[:, e, ct * 128:(ct + 1) * 128],
                                xT[:, e, tcn * 512:(tcn + 1) * 512],
                                start=(e == 0), stop=(e == 7),
                            )
                        nc.any.tensor_copy(
                            dstT[:, ct, tcn * 512:(tcn + 1) * 512], ps
                        )
            for tb in range(16):
                ps = prp.tile([128, CPC], F32, tag="ps")
                for e in range(8):
                    nc.tensor.matmul(
                        ps,
                        xT[:, e, tb * 128:(tb + 1) * 128],
                        WvT[:, e, :],
                        start=(e == 0), stop=(e == 7),
                    )
                nc.any.tensor_copy(V[:, tb, :], ps)

        # ---------------- phase 2: attention, qgroup-major ----------------
        ctx2 = ctx.enter_context(ExitStack())
        epool = ctx2.enter_context(tc.tile_pool(name="epool", bufs=2))
        t1pool = ctx2.enter_context(tc.tile_pool(name="t1pool", bufs=2))
        apool = ctx2.enter_context(tc.tile_pool(name="apool", bufs=2))
        abpool = ctx2.enter_context(tc.tile_pool(name="abpool", bufs=2))
        atpool = ctx2.enter_context(tc.tile_pool(name="atpool", bufs=2))
        dpool = ctx2.enter_context(tc.tile_pool(name="dpool", bufs=3))
        spool = ctx2.enter_context(tc.tile_pool(name="spool", bufs=3))
        rmssb = ctx2.enter_context(tc.tile_pool(name="rmssb", bufs=2))
        oop = ctx2.enter_context(tc.tile_pool(name="oout", bufs=3))
        spsum = ctx2.enter_context(tc.tile_pool(name="spsum", bufs=1, space="PSUM"))
        tpsum = ctx2.enter_context(tc.tile_pool(name="tpsum", bufs=1, space="PSUM"))
        avpsum = ctx2.enter_context(tc.tile_pool(name="avpsum", bufs=1, space="PSUM"))
        smpsum = ctx2.enter_context(tc.tile_pool(name="smpsum", bufs=1, space="PSUM"))
        opsum = ctx2.enter_context(tc.tile_pool(name="opsum", bufs=1, space="PSUM"))

        for qg in range(4):
            for pair in range(HPC):
                tpi = pair // 2
                b1 = (pair % 2) * 64
                b2 = b1 + 32
                att = atpool.tile([128, 16, 512], BF16, tag="AT")
                for qb in range(qg * 4, qg * 4 + 4):
                    lk = (qb + 1) * 128
                    e1 = epool.tile([128, L], F32, tag="E1")
                    e2 = epool.tile([128, L], F32, tag="E2")
                    db = dpool.tile([128, 8], F32, tag="db")
                    nkc = (lk + 1023) // 1024
                    for kc in range(nkc):
                        w = min(1024, lk - kc * 1024)
                        s1 = spsum.tile([128, 1024], F32, tag="S1")
                        s2 = spsum.tile([128, 1024], F32, tag="S2")
                        for st, b in ((s1, b1), (s2, b2)):
                            for half in range(0, w, 512):
                                hw = min(512, w - half)
                                nc.tensor.matmul(
                                    st[:, half:half + hw],
                                    QT[b:b + 32, tpi, qb * 128:(qb + 1) * 128],
                                    KT[b:b + 32, tpi,
                                       kc * 1024 + half:kc * 1024 + half + hw],
                                    start=True, stop=True, tile_position=(b, 0),
                                )
                            if kc == nkc - 1:
                                nc.tensor.matmul(
                                    st[:, w - 128:w], mdiagT, identb,
                                    start=False, stop=True,
                                )
                        nc.scalar.activation(
                            e1[:, kc * 1024:kc * 1024 + w], s1[:, :w], AF.Exp,
                            scale=SCALE, accum_out=db[:, kc:kc + 1],
                        )
                        nc.scalar.activation(
                            e2[:, kc * 1024:kc * 1024 + w], s2[:, :w], AF.Exp,
                            scale=SCALE, accum_out=db[:, 4 + kc:4 + kc + 1],
                        )
                    ds = spool.tile([128, 2], F32, tag="ds")
                    nc.vector.reduce_sum(
                        ds,
                        db.rearrange("p (h k) -> p h k", h=2)[:, :, 0:nkc],
                        axis=mybir.AxisListType.X,
                    )
                    rs = spool.tile([128, 2], F32, tag="rs")
                    nc.vector.reciprocal(rs, ds)
                    gf = spool.tile([128, 2], F32, tag="gf")  # col0=g, col1=f2
                    nc.vector.scalar_tensor_tensor(
                        out=gf[:, 0:1], in0=ds[:, 1:2], scalar=laminv_ap,
                        in1=rs[:, 0:1], op0=OP.mult, op1=OP.mult,
                    )
                    nc.vector.tensor_scalar_mul(gf[:, 1:2], rs[:, 1:2], lam_ap)
                    t1 = t1pool.tile([128, L], F32, tag="T1")
                    nc.vector.scalar_tensor_tensor(
                        out=t1[:, :lk], in0=e1[:, :lk], scalar=gf[:, 0:1],
                        in1=e2[:, :lk], op0=OP.mult, op1=OP.subtract,
                    )
                    aw = apool.tile([128, L], F32, tag="A")
                    nc.vector.tensor_scalar_mul(aw[:, :lk], t1[:, :lk], gf[:, 1:2])
                    nc.sync.dma_start(
                        out=a_d[pair, qb * 128:(qb + 1) * 128, 0:lk],
                        in_=aw[:, :lk],
                    )
                    # bf16 copy of A (scaled) feeding the transposes
                    ab = abpool.tile([128, L], BF16, tag="Ab")
                    nc.vector.tensor_scalar_mul(ab[:, :lk], t1[:, :lk], gf[:, 1:2])
                    qc = (qb % 4) * 128
                    for g8 in range((qb + 8) // 8):
                        nb = min(8, qb + 1 - g8 * 8)
                        tp = tpsum.tile([128, 8, 128], BF16, tag="tp")
                        for j in range(nb):
                            kb = g8 * 8 + j
                            nc.tensor.transpose(
                                tp[:, j, :], ab[:, kb * 128:(kb + 1) * 128],
                                identb,
                            )
                        nc.any.tensor_copy(
                            att[:, g8 * 8:g8 * 8 + nb, qc:qc + 128],
                            tp[:, 0:nb, :],
                        )
                # A@V for this (pair, qgroup)
                ot = avpsum.tile([64, 512], F32, tag="ot")
                nkb = (qg + 1) * 4
                for kb in range(nkb):
                    c0 = max(0, kb * 128 - qg * 512)
                    nc.tensor.matmul(
                        ot[:, c0:512],
                        V[:, kb, pair * 64:(pair + 1) * 64],
                        att[:, kb, c0:512],
                        start=(kb == 0), stop=(kb == nkb - 1),
                    )
                nc.any.tensor_copy(OTsb[:, pair, qg * 512:(qg + 1) * 512], ot)
                # rms statistic
                sq = rmssb.tile([64, 512], BF16, tag="sq")
                osl = OTsb[:, pair, qg * 512:(qg + 1) * 512]
                nc.vector.tensor_mul(sq, osl, osl)
                ms = smpsum.tile([1, 512], F32, tag="sm")
                nc.tensor.matmul(ms, ones64, sq, start=True, stop=True)
                mss = rmssb.tile([1, 512], F32, tag="mss")
                nc.any.tensor_copy(mss, ms)
                nc.sync.dma_start(out=ms4[pair:pair + 1, :], in_=mss)
            # per-qgroup rms finish + o_proj
            rt4 = rmssb.tile([4, 512], F32, tag="rt4")
            nc.scalar.activation(rt4, ms4, AF.Sqrt, bias=eps4)
            inv4 = rmssb.tile([4, 512], F32, tag="inv4")
            sc4 = rmssb.tile([4, 512], F32, tag="sc4")
            nc.vector.reciprocal_approx_accurate(inv4, rt4, sc4)
            for pair in range(HPC):
                bc = smpsum.tile([64, 512], F32, tag="sm")
                nc.tensor.matmul(
                    bc, c3sel4[:, pair * 64:(pair + 1) * 64], inv4,
                    start=True, stop=True,
                )
                nc.vector.tensor_mul(
                    OTf[:, pair, qg * 512:(qg + 1) * 512],
                    OTsb[:, pair, qg * 512:(qg + 1) * 512], bc,
                )
            for eo in range(2):
                for tb in range(qg * 4, qg * 4 + 4):
                    po = opsum.tile([128, 512], F32, tag="po")
                    for p in range(HPC):
                        nc.tensor.matmul(
                            po,
                            OTf[:, p, tb * 128:(tb + 1) * 128],
                            WoT[:, p, eo * 512:(eo + 1) * 512],
                            start=(p == 0), stop=(p == 3),
                        )
                    ob = oop.tile([128, 512], F32, tag="ob")
                    nc.any.tensor_copy(ob, po)
                    nc.sync.dma_start(
                        out=o_d[tb * 128:(tb + 1) * 128,
                                eo * 512:(eo + 1) * 512],
                        in_=ob,
                    )

    nc.finalize()
    _CACHE["nc"] = nc
    return nc


def kernel(x, Wq, Wk, Wv, Wo, lambda_q1, lambda_k1, lambda_q2, lambda_k2):
    global LAST_EXEC_NS
    x = np.asarray(x, np.float32)
    Wq = np.asarray(Wq, np.float32)
    Wk = np.asarray(Wk, np.float32)
    Wv = np.asarray(Wv, np.float32)
    Wo = np.asarray(Wo, np.float32)

    nc = _build()
    in_maps = []
    for c in range(N_CORES):
        b, g = divmod(c, HPC)
        s = slice(g * CPC, (g + 1) * CPC)
        in_maps.append({
            "x": np.ascontiguousarray(x[b]),
            "wq": np.ascontiguousarray(Wq[s, :]),
            "wk": np.ascontiguousarray(Wk[s, :]),
            "wv": np.ascontiguousarray(Wv[s, :]),
            "wo": np.ascontiguousarray(Wo[:, s]),
            "lq1": np.asarray(lambda_q1, np.float32),
            "lk1": np.asarray(lambda_k1, np.float32),
            "lq2": np.asarray(lambda_q2, np.float32),
            "lk2": np.asarray(lambda_k2, np.float32),
        })

    res = run_bass_kernel_spmd(
        nc, in_maps, core_ids=list(range(N_CORES)), trace=TRACE
    )
    LAST_EXEC_NS = res.exec_time_ns

    A = np.empty((B, NH, L, L), np.float32)
    O = np.zeros((B, L, E), np.float32)
    for c in range(N_CORES):
        b, g = divmod(c, HPC)
        A[b, g * HPC:(g + 1) * HPC] = res.results[c]["A_out"]
        O[b] += res.results[c]["O_out"]
    return O, A
